# revision 15
# baseline (speedup 1.0000x reference)
"""Trainium2 Bass kernel for nn_Attention_11287174054323.

Full attention layer: QKV projections + RoPE + softmax attention + output
projection.  B=2, S=2048, DIM=2048, 16 heads x 128 head_dim, fp32.

Sharding: tensor-parallel over heads across 8 NeuronCores (2 heads/core).
Each core computes q/k/v projections for its head slice, full attention for
its heads, and a partial output projection (row slice of Wo); the host sums
the 8 partials.

Per-core layout strategy:
  - x is passed pre-transposed (xT [DIM, B*S]) so projections can contract
    over DIM on the partition axis.
  - Q^T/K^T are produced in [head_dim, token] layout; RoPE is fused into the
    PSUM eviction (rotate-half via cross-partition-write multiplies).
  - Scores are computed transposed (S^T = K @ Q^T), softmax-normalization is
    deferred past the A^T = V^T @ exp(S^T) accumulation (linearity); the
    denominator is accumulated with M=1 ones-matmuls in PSUM, broadcast with
    a K=1 matmul, inverted with reciprocal_approx_fast.
  - Matmuls run in float32r (tf32-like, ~13-bit mantissa, 4x fp32 rate).
  - PSUM->SBUF evictions that would crowd the vector engine go to the scalar
    engine (ACT) in phases where it is otherwise idle.
"""

import sys

sys.path.insert(0, "/opt/trn_rl_repo")

import numpy as np

import concourse.tile as tile
import concourse.mybir as mybir
from concourse import bacc
from concourse.bass_utils import run_bass_kernel_spmd

P = 128
B, S, DIM = 2, 2048, 2048
TOK = B * S                     # 4096 tokens
HEADS_PER_CORE = 2
INNER_C = HEADS_PER_CORE * P    # 256 per-core inner dim
KC = DIM // P                   # 16 contraction chunks
TC = 512                        # phase-1 token chunk
NTC = TOK // TC                 # 8
IC = 512                        # attention i-chunk (queries)
NIC = S // IC                   # 4 per (batch, head)
NJC = S // P                    # 16 key chunks per (batch, head)
SCALE = float(P) ** -0.5

F32 = mybir.dt.float32
MM = mybir.dt.float32           # matmul dtype (float32r or float32)

N_CORES = 8
Mul = mybir.AluOpType.mult


def _build():
    nc = bacc.Bacc("TRN2", target_bir_lowering=False)

    xT_d = nc.dram_tensor("xT", [DIM, TOK], MM, kind="ExternalInput")
    wq_d = nc.dram_tensor("wq", [DIM, INNER_C], MM, kind="ExternalInput")
    wk_d = nc.dram_tensor("wk", [DIM, INNER_C], MM, kind="ExternalInput")
    wv_d = nc.dram_tensor("wv", [DIM, INNER_C], MM, kind="ExternalInput")
    wo_d = nc.dram_tensor("wo", [INNER_C, DIM], MM, kind="ExternalInput")
    cos_d = nc.dram_tensor("cosT", [P, TOK], F32, kind="ExternalInput")
    sin_d = nc.dram_tensor("sinZ", [P, TOK], F32, kind="ExternalInput")
    ones_d = nc.dram_tensor("ones", [P, P], MM, kind="ExternalInput")
    o_d = nc.dram_tensor("o_part", [TOK, DIM], F32, kind="ExternalOutput")

    xT_r = xT_d.rearrange("(c p) t -> p c t", p=P)

    with tile.TileContext(nc) as tc:
        with tc.tile_pool(name="persist", bufs=1) as persist, \
             tc.tile_pool(name="dram", bufs=1, space="DRAM") as dram:
            qt = [persist.tile([P, TOK], MM, tag=f"qt{h}", name=f"qt{h}")
                  for h in range(2)]
            kt = [persist.tile([P, TOK], MM, tag=f"kt{h}", name=f"kt{h}")
                  for h in range(2)]
            ones_t = persist.tile([P, P], MM, tag="ones")
            nc.sync.dma_start(ones_t[:], ones_d[:])
            v_dram = dram.tile([TOK, INNER_C], MM)

            # ---------------- Phase 1: QKV projections + RoPE ----------
            with tc.tile_pool(name="w1", bufs=1) as wpool, \
                 tc.tile_pool(name="x1", bufs=2) as xpool, \
                 tc.tile_pool(name="ev1", bufs=2) as evpool, \
                 tc.tile_pool(name="ps1", bufs=1, space="PSUM") as psum1:
                wq_t = wpool.tile([P, KC, INNER_C], MM, tag="wq")
                wk_t = wpool.tile([P, KC, INNER_C], MM, tag="wk")
                wv_t = wpool.tile([P, KC, INNER_C], MM, tag="wv")
                nc.sync.dma_start(wq_t[:], wq_d.rearrange("(c p) m -> p c m", p=P))
                nc.sync.dma_start(wk_t[:], wk_d.rearrange("(c p) m -> p c m", p=P))
                nc.sync.dma_start(wv_t[:], wv_d.rearrange("(c p) m -> p c m", p=P))

                for tcn in range(NTC):
                    tsl = slice(tcn * TC, (tcn + 1) * TC)
                    xt = xpool.tile([P, KC, TC], MM, tag="xt")
                    nc.sync.dma_start(xt[:], xT_r[:, :, tsl])
                    cos_t = evpool.tile([P, TC], F32, tag="cos")
                    sin_t = evpool.tile([P, TC], F32, tag="sin")
                    nc.sync.dma_start(cos_t[:], cos_d[:, tsl])
                    nc.sync.dma_start(sin_t[:], sin_d[:, tsl])

                    # Q^T / K^T chunks with fused RoPE eviction
                    for wt, dsts, nm in ((wq_t, qt, "q"), (wk_t, kt, "k")):
                        for m in range(2):
                            ps = psum1.tile([P, TC], F32, tag=f"ps_{nm}{m}")
                            for kc in range(KC):
                                nc.tensor.matmul(
                                    ps[:], wt[:, kc, m * P:(m + 1) * P],
                                    xt[:, kc, :],
                                    start=(kc == 0), stop=(kc == KC - 1))
                            # rope: dst = ps*cos + rotate_half(ps)*sin
                            tcos = evpool.tile([P, TC], F32, tag="tcos")
                            nc.vector.tensor_mul(tcos[:], ps[:], cos_t[:])
                            tsin = evpool.tile([P, TC], F32, tag="tsin")
                            nc.vector.scalar_tensor_tensor(
                                tsin[0:64, :], ps[64:128, :], 1.0,
                                sin_t[64:128, :], Mul, Mul)
                            nc.vector.scalar_tensor_tensor(
                                tsin[64:128, :], ps[0:64, :], 1.0,
                                sin_t[0:64, :], Mul, Mul)
                            nc.vector.tensor_add(dsts[m][:, tsl], tcos[:],
                                                 tsin[:])

                    # V chunks (tokens on partitions) -> DRAM scratch
                    for m in range(TC // P):
                        ps = psum1.tile([P, INNER_C], F32, tag=f"ps_v{m}")
                        for kc in range(KC):
                            nc.tensor.matmul(
                                ps[:], xt[:, kc, m * P:(m + 1) * P],
                                wv_t[:, kc, :],
                                start=(kc == 0), stop=(kc == KC - 1))
                        vst = evpool.tile([P, INNER_C], MM, tag="vst")
                        nc.scalar.copy(vst[:], ps[:])
                        r0 = tcn * TC + m * P
                        nc.sync.dma_start(v_dram[r0:r0 + P, :], vst[:])

            # ---------- Phases 2+3: attention + output projection ------
            with tc.tile_pool(name="at", bufs=1) as atpool, \
                 tc.tile_pool(name="vbh", bufs=2) as vpool, \
                 tc.tile_pool(name="e2", bufs=4) as epool, \
                 tc.tile_pool(name="sm2", bufs=2) as smpool, \
                 tc.tile_pool(name="p3", bufs=1) as p3pool, \
                 tc.tile_pool(name="st3", bufs=3) as stpool, \
                 tc.tile_pool(name="ps2", bufs=1, space="PSUM") as psum2, \
                 tc.tile_pool(name="ps3", bufs=2, space="PSUM") as psum3:
                at = [atpool.tile([P, TOK], MM, tag=f"at{h}", name=f"at{h}")
                      for h in range(2)]
                wo_t = p3pool.tile([P, 2, DIM], MM, tag="wo")
                nc.sync.dma_start(wo_t[:],
                                  wo_d.rearrange("(h p) e -> p h e", p=P))

                def load_vbh(b, h):
                    boff = b * S
                    vbh = vpool.tile([P, NJC, P], MM, tag="vbh")
                    nc.sync.dma_start(
                        vbh[:],
                        v_dram[boff:boff + S, h * P:(h + 1) * P]
                        .rearrange("(c p) d -> p c d", p=P))
                    return vbh

                def attn_ic(b, h, icn, vbh):
                    """One 512-query chunk of attention for (batch, head)."""
                    boff = b * S
                    isl = slice(boff + icn * IC, boff + (icn + 1) * IC)
                    ps_at = psum2.tile([P, IC], F32, tag="ps_at", bufs=2)
                    acc = smpool.tile([P, IC], MM, tag="acc")
                    # software-pipelined S -> exp -> (A, colsum) chain
                    es = [None] * NJC

                    def s_step(jc):
                        jsl = slice(boff + jc * P, boff + (jc + 1) * P)
                        ps_s = psum2.tile([P, IC], F32, tag="ps_s", bufs=2)
                        nc.tensor.matmul(ps_s[:], kt[h][:, jsl], qt[h][:, isl],
                                         start=True, stop=True)
                        e = epool.tile([P, IC], MM, tag="e")
                        nc.scalar.activation(
                            e[:], ps_s[:], mybir.ActivationFunctionType.Exp,
                            scale=SCALE)
                        es[jc] = e

                    def a_step(jc):
                        e = es[jc]
                        nc.tensor.matmul(ps_at[:], vbh[:, jc, :], e[:],
                                         start=(jc == 0), stop=(jc == NJC - 1))
                        if jc == 0:
                            nc.vector.tensor_copy(acc[:], e[:].bitcast(F32))
                        else:
                            nc.vector.tensor_add(acc[:], acc[:].bitcast(F32),
                                                 e[:].bitcast(F32))

                    s_step(0)
                    for jc in range(NJC):
                        if jc + 1 < NJC:
                            s_step(jc + 1)
                        a_step(jc)

                    ps_bc = psum2.tile([P, IC], F32, tag="ps_bc", bufs=1)
                    nc.tensor.matmul(ps_bc[:], ones_t[:], acc[:],
                                     start=True, stop=True)
                    recip = smpool.tile([P, IC], F32, tag="recip")
                    nc.vector.reciprocal_approx_fast(recip[:], ps_bc[:])
                    nc.vector.tensor_mul(at[h][:, isl], ps_at[:], recip[:])

                def ph3_tn(tn):
                    """One 128-token chunk of the output projection."""
                    stage = stpool.tile([P, DIM], F32, tag="stage")
                    for en in range(DIM // IC):
                        ps = psum3.tile([P, IC], F32, tag="ps_o")
                        esl = slice(en * IC, (en + 1) * IC)
                        for h in range(2):
                            nc.tensor.matmul(
                                ps[:], at[h][:, tn * P:(tn + 1) * P],
                                wo_t[:, h, esl],
                                start=(h == 0), stop=(h == 1))
                        nc.scalar.copy(stage[:, esl], ps[:])
                    nc.sync.dma_start(o_d[tn * P:(tn + 1) * P, :], stage[:])

                # batch 0 attention
                for h in range(2):
                    vbh = load_vbh(0, h)
                    for icn in range(NIC):
                        attn_ic(0, h, icn, vbh)
                # batch 1 attention interleaved with batch-0 out-projection
                done3 = 0
                for h in range(2):
                    vbh = load_vbh(1, h)
                    for icn in range(NIC):
                        attn_ic(1, h, icn, vbh)
                        ph3_tn(done3)
                        ph3_tn(done3 + 1)
                        done3 += 2
                # batch-1 out-projection
                for tn in range(S // P):
                    ph3_tn(done3 + tn)

    nc.finalize()
    return nc


def _rope_tables():
    """cos/sin tables in [head_dim, token] layout, matching the reference's
    f32 computation (jax on CPU when available).

    sinZ rows 0:64 hold +sin (multiplied against q[d-64] to produce rows
    64:128 of the rotation term) and rows 64:128 hold -sin (multiplied
    against q[d+64] to produce rows 0:64); both halves of the underlying
    sin table are identical (emb = concat(freqs, freqs)).
    """
    try:
        import jax
        import jax.numpy as jnp
        cpu = jax.devices("cpu")[0]
        with jax.default_device(cpu):
            inv = 1.0 / (10000.0 ** (
                jnp.arange(0, P, 2, dtype=jnp.float32) / P))
            t = jnp.arange(S, dtype=jnp.float32)
            freqs = jnp.einsum("i,j->ij", t, inv)          # [S, 64]
            emb = jnp.concatenate((freqs, freqs), axis=-1)  # [S, 128]
            cos = np.asarray(jnp.cos(emb)).T                # [128, S]
            sin = np.asarray(jnp.sin(emb)).T
    except Exception:
        inv = 1.0 / (10000.0 ** (np.arange(0, P, 2, dtype=np.float64) / P))
        t = np.arange(S, dtype=np.float64)
        freqs = np.outer(t, inv)
        emb = np.concatenate((freqs, freqs), axis=-1)
        cos = np.cos(emb).T.astype(np.float32)
        sin = np.sin(emb).T.astype(np.float32)

    cos2 = np.ascontiguousarray(np.tile(cos, (1, B)).astype(np.float32))
    sin_z = np.concatenate([sin[0:64], -sin[64:128]], axis=0)
    sin2 = np.ascontiguousarray(np.tile(sin_z, (1, B)).astype(np.float32))
    return cos2, sin2


_NC_CACHE = None


def _in_maps(x, Wq, Wk, Wv, Wo):
    xT = np.ascontiguousarray(x.reshape(TOK, DIM).T).astype(np.float32)
    cosT, sinZ = _rope_tables()
    ones = np.ones((P, P), dtype=np.float32)
    maps = []
    for c in range(N_CORES):
        cs = slice(c * INNER_C, (c + 1) * INNER_C)
        maps.append({
            "xT": xT,
            "wq": np.ascontiguousarray(Wq[:, cs]).astype(np.float32),
            "wk": np.ascontiguousarray(Wk[:, cs]).astype(np.float32),
            "wv": np.ascontiguousarray(Wv[:, cs]).astype(np.float32),
            "wo": np.ascontiguousarray(Wo[cs, :]).astype(np.float32),
            "cosT": cosT,
            "sinZ": sinZ,
            "ones": ones,
        })
    return maps


def kernel(x, Wq, Wk, Wv, Wo):
    global _NC_CACHE
    assert x.shape == (B, S, DIM)
    if _NC_CACHE is None:
        _NC_CACHE = _build()
    res = run_bass_kernel_spmd(_NC_CACHE, _in_maps(x, Wq, Wk, Wv, Wo),
                               core_ids=list(range(N_CORES)), trace=False)
    out = res.results[0]["o_part"].astype(np.float64)
    for c in range(1, N_CORES):
        out += res.results[c]["o_part"]
    return out.astype(np.float32).reshape(B, S, DIM)


# revision 17
# speedup vs baseline: 2.7408x; 2.7408x over previous
"""Trainium2 Bass kernel for nn_Attention_11287174054323.

Full attention layer: QKV projections + RoPE + softmax attention + output
projection.  B=2, S=2048, DIM=2048, 16 heads x 128 head_dim, fp32.

Sharding: tensor-parallel over heads across 8 NeuronCores (2 heads/core).
Each core computes q/k/v projections for its head slice, full attention for
its heads, and a partial output projection (row slice of Wo); the host sums
the 8 partials.

Per-core layout strategy:
  - x is passed pre-transposed (xT [DIM, B*S]) so projections can contract
    over DIM on the partition axis.
  - Q^T/K^T are produced in [head_dim, token] layout; RoPE is fused into the
    PSUM eviction (rotate-half via cross-partition-write multiplies).
  - Scores are computed transposed (S^T = K @ Q^T), softmax-normalization is
    deferred past the A^T = V^T @ exp(S^T) accumulation (linearity); the
    denominator is accumulated with M=1 ones-matmuls in PSUM, broadcast with
    a K=1 matmul, inverted with reciprocal_approx_fast.
  - Matmuls run in float32r (tf32-like, ~13-bit mantissa, 4x fp32 rate).
  - PSUM->SBUF evictions that would crowd the vector engine go to the scalar
    engine (ACT) in phases where it is otherwise idle.
"""

import sys

sys.path.insert(0, "/opt/trn_rl_repo")

import numpy as np

import concourse.tile as tile
import concourse.mybir as mybir
from concourse import bacc
from concourse.bass_utils import run_bass_kernel_spmd

P = 128
B, S, DIM = 2, 2048, 2048
TOK = B * S                     # 4096 tokens
HEADS_PER_CORE = 2
INNER_C = HEADS_PER_CORE * P    # 256 per-core inner dim
KC = DIM // P                   # 16 contraction chunks
TC = 512                        # phase-1 token chunk
NTC = TOK // TC                 # 8
IC = 512                        # attention i-chunk (queries)
NIC = S // IC                   # 4 per (batch, head)
NJC = S // P                    # 16 key chunks per (batch, head)
SCALE = float(P) ** -0.5

F32 = mybir.dt.float32
MM = mybir.dt.float32r          # matmul dtype (float32r or float32)

N_CORES = 8
Mul = mybir.AluOpType.mult


def _build():
    nc = bacc.Bacc("TRN2", target_bir_lowering=False)

    xT_d = nc.dram_tensor("xT", [DIM, TOK], MM, kind="ExternalInput")
    wq_d = nc.dram_tensor("wq", [DIM, INNER_C], MM, kind="ExternalInput")
    wk_d = nc.dram_tensor("wk", [DIM, INNER_C], MM, kind="ExternalInput")
    wv_d = nc.dram_tensor("wv", [DIM, INNER_C], MM, kind="ExternalInput")
    wo_d = nc.dram_tensor("wo", [INNER_C, DIM], MM, kind="ExternalInput")
    cos_d = nc.dram_tensor("cosT", [P, TOK], F32, kind="ExternalInput")
    sin_d = nc.dram_tensor("sinZ", [P, TOK], F32, kind="ExternalInput")
    ones_d = nc.dram_tensor("ones", [P, P], MM, kind="ExternalInput")
    o_d = nc.dram_tensor("o_part", [TOK, DIM], F32, kind="ExternalOutput")

    xT_r = xT_d.rearrange("(c p) t -> p c t", p=P)

    with tile.TileContext(nc) as tc:
        with tc.tile_pool(name="persist", bufs=1) as persist, \
             tc.tile_pool(name="dram", bufs=1, space="DRAM") as dram:
            qt = [persist.tile([P, TOK], MM, tag=f"qt{h}", name=f"qt{h}")
                  for h in range(2)]
            kt = [persist.tile([P, TOK], MM, tag=f"kt{h}", name=f"kt{h}")
                  for h in range(2)]
            ones_t = persist.tile([P, P], MM, tag="ones")
            nc.sync.dma_start(ones_t[:], ones_d[:])
            v_dram = dram.tile([TOK, INNER_C], MM)

            # ---------------- Phase 1: QKV projections + RoPE ----------
            with tc.tile_pool(name="w1", bufs=1) as wpool, \
                 tc.tile_pool(name="x1", bufs=2) as xpool, \
                 tc.tile_pool(name="ev1", bufs=2) as evpool, \
                 tc.tile_pool(name="ps1", bufs=1, space="PSUM") as psum1:
                wq_t = wpool.tile([P, KC, INNER_C], MM, tag="wq")
                wk_t = wpool.tile([P, KC, INNER_C], MM, tag="wk")
                wv_t = wpool.tile([P, KC, INNER_C], MM, tag="wv")
                # critical path first: wq + chunk-0 activations, then wk/wv
                nc.sync.dma_start(wq_t[:], wq_d.rearrange("(c p) m -> p c m", p=P))
                xt0 = xpool.tile([P, KC, TC], MM, tag="xt")
                nc.sync.dma_start(xt0[:], xT_r[:, :, 0:TC])
                cos0 = evpool.tile([P, TC], F32, tag="cos")
                sin0 = evpool.tile([P, TC], F32, tag="sin")
                nc.sync.dma_start(cos0[:], cos_d[:, 0:TC])
                nc.sync.dma_start(sin0[:], sin_d[:, 0:TC])
                nc.sync.dma_start(wk_t[:], wk_d.rearrange("(c p) m -> p c m", p=P))
                nc.sync.dma_start(wv_t[:], wv_d.rearrange("(c p) m -> p c m", p=P))

                for tcn in range(NTC):
                    tsl = slice(tcn * TC, (tcn + 1) * TC)
                    if tcn == 0:
                        xt, cos_t, sin_t = xt0, cos0, sin0
                    else:
                        xt = xpool.tile([P, KC, TC], MM, tag="xt")
                        nc.sync.dma_start(xt[:], xT_r[:, :, tsl])
                        cos_t = evpool.tile([P, TC], F32, tag="cos")
                        sin_t = evpool.tile([P, TC], F32, tag="sin")
                        nc.sync.dma_start(cos_t[:], cos_d[:, tsl])
                        nc.sync.dma_start(sin_t[:], sin_d[:, tsl])

                    # Q^T / K^T chunks with fused RoPE eviction
                    for wt, dsts, nm in ((wq_t, qt, "q"), (wk_t, kt, "k")):
                        for m in range(2):
                            ps = psum1.tile([P, TC], F32, tag=f"ps_{nm}{m}")
                            for kc in range(KC):
                                nc.tensor.matmul(
                                    ps[:], wt[:, kc, m * P:(m + 1) * P],
                                    xt[:, kc, :],
                                    start=(kc == 0), stop=(kc == KC - 1))
                            # rope: dst = ps*cos + rotate_half(ps)*sin
                            tcos = evpool.tile([P, TC], F32, tag="tcos")
                            nc.vector.tensor_mul(tcos[:], ps[:], cos_t[:])
                            tsin = evpool.tile([P, TC], F32, tag="tsin")
                            nc.vector.scalar_tensor_tensor(
                                tsin[0:64, :], ps[64:128, :], 1.0,
                                sin_t[64:128, :], Mul, Mul)
                            nc.vector.scalar_tensor_tensor(
                                tsin[64:128, :], ps[0:64, :], 1.0,
                                sin_t[0:64, :], Mul, Mul)
                            nc.vector.tensor_add(dsts[m][:, tsl], tcos[:],
                                                 tsin[:])

                    # V chunks (tokens on partitions) -> DRAM scratch
                    for m in range(TC // P):
                        ps = psum1.tile([P, INNER_C], F32, tag=f"ps_v{m}")
                        for kc in range(KC):
                            nc.tensor.matmul(
                                ps[:], xt[:, kc, m * P:(m + 1) * P],
                                wv_t[:, kc, :],
                                start=(kc == 0), stop=(kc == KC - 1))
                        vst = evpool.tile([P, INNER_C], MM, tag="vst")
                        nc.scalar.copy(vst[:], ps[:])
                        r0 = tcn * TC + m * P
                        nc.sync.dma_start(v_dram[r0:r0 + P, :], vst[:])

            # ---------- Phases 2+3: attention + output projection ------
            with tc.tile_pool(name="at", bufs=1) as atpool, \
                 tc.tile_pool(name="vbh", bufs=2) as vpool, \
                 tc.tile_pool(name="e2", bufs=4) as epool, \
                 tc.tile_pool(name="sm2", bufs=2) as smpool, \
                 tc.tile_pool(name="p3", bufs=1) as p3pool, \
                 tc.tile_pool(name="st3", bufs=3) as stpool, \
                 tc.tile_pool(name="ps2", bufs=1, space="PSUM") as psum2, \
                 tc.tile_pool(name="ps3", bufs=2, space="PSUM") as psum3:
                at = [atpool.tile([P, TOK], MM, tag=f"at{h}", name=f"at{h}")
                      for h in range(2)]
                wo_t = p3pool.tile([P, 2, DIM], MM, tag="wo")
                nc.sync.dma_start(wo_t[:],
                                  wo_d.rearrange("(h p) e -> p h e", p=P))

                def load_vbh(b, h):
                    boff = b * S
                    vbh = vpool.tile([P, NJC, P], MM, tag="vbh")
                    nc.sync.dma_start(
                        vbh[:],
                        v_dram[boff:boff + S, h * P:(h + 1) * P]
                        .rearrange("(c p) d -> p c d", p=P))
                    return vbh

                def attn_ic(b, h, icn, vbh):
                    """One 512-query chunk of attention for (batch, head)."""
                    boff = b * S
                    isl = slice(boff + icn * IC, boff + (icn + 1) * IC)
                    ps_at = psum2.tile([P, IC], F32, tag="ps_at", bufs=2)
                    acc = smpool.tile([P, IC], MM, tag="acc")
                    # software-pipelined S -> exp -> (A, colsum) chain
                    es = [None] * NJC

                    def s_step(jc):
                        jsl = slice(boff + jc * P, boff + (jc + 1) * P)
                        ps_s = psum2.tile([P, IC], F32, tag="ps_s", bufs=2)
                        nc.tensor.matmul(ps_s[:], kt[h][:, jsl], qt[h][:, isl],
                                         start=True, stop=True)
                        e = epool.tile([P, IC], MM, tag="e")
                        nc.scalar.activation(
                            e[:], ps_s[:], mybir.ActivationFunctionType.Exp,
                            scale=SCALE)
                        es[jc] = e

                    def a_step(jc):
                        e = es[jc]
                        nc.tensor.matmul(ps_at[:], vbh[:, jc, :], e[:],
                                         start=(jc == 0), stop=(jc == NJC - 1))
                        if jc == 0:
                            nc.vector.tensor_copy(acc[:], e[:].bitcast(F32))
                        else:
                            nc.vector.tensor_add(acc[:], acc[:].bitcast(F32),
                                                 e[:].bitcast(F32))

                    s_step(0)
                    for jc in range(NJC):
                        if jc + 1 < NJC:
                            s_step(jc + 1)
                        a_step(jc)

                    ps_bc = psum2.tile([P, IC], F32, tag="ps_bc", bufs=1)
                    nc.tensor.matmul(ps_bc[:], ones_t[:], acc[:],
                                     start=True, stop=True)
                    recip = smpool.tile([P, IC], F32, tag="recip")
                    nc.vector.reciprocal_approx_fast(recip[:], ps_bc[:])
                    nc.vector.tensor_mul(at[h][:, isl], ps_at[:], recip[:])

                def ph3_tn(tn):
                    """One 128-token chunk of the output projection."""
                    stage = stpool.tile([P, DIM], F32, tag="stage")
                    for en in range(DIM // IC):
                        ps = psum3.tile([P, IC], F32, tag="ps_o")
                        esl = slice(en * IC, (en + 1) * IC)
                        for h in range(2):
                            nc.tensor.matmul(
                                ps[:], at[h][:, tn * P:(tn + 1) * P],
                                wo_t[:, h, esl],
                                start=(h == 0), stop=(h == 1))
                        nc.scalar.copy(stage[:, esl], ps[:])
                    nc.sync.dma_start(o_d[tn * P:(tn + 1) * P, :], stage[:])

                # batch 0 attention
                for h in range(2):
                    vbh = load_vbh(0, h)
                    for icn in range(NIC):
                        attn_ic(0, h, icn, vbh)
                # batch 1 attention; interleave batch-0 out-projection during
                # h=0 and batch-1 chunks (as their at-slices complete) in h=1
                vbh = load_vbh(1, 0)
                for icn in range(NIC):
                    attn_ic(1, 0, icn, vbh)
                    for k in range(4):
                        ph3_tn(icn * 4 + k)
                vbh = load_vbh(1, 1)
                for icn in range(NIC):
                    attn_ic(1, 1, icn, vbh)
                    for k in range(4):
                        ph3_tn(16 + icn * 4 + k)

    nc.finalize()
    return nc


def _rope_tables():
    """cos/sin tables in [head_dim, token] layout, matching the reference's
    f32 computation (jax on CPU when available).

    sinZ rows 0:64 hold +sin (multiplied against q[d-64] to produce rows
    64:128 of the rotation term) and rows 64:128 hold -sin (multiplied
    against q[d+64] to produce rows 0:64); both halves of the underlying
    sin table are identical (emb = concat(freqs, freqs)).
    """
    try:
        import jax
        import jax.numpy as jnp
        cpu = jax.devices("cpu")[0]
        with jax.default_device(cpu):
            inv = 1.0 / (10000.0 ** (
                jnp.arange(0, P, 2, dtype=jnp.float32) / P))
            t = jnp.arange(S, dtype=jnp.float32)
            freqs = jnp.einsum("i,j->ij", t, inv)          # [S, 64]
            emb = jnp.concatenate((freqs, freqs), axis=-1)  # [S, 128]
            cos = np.asarray(jnp.cos(emb)).T                # [128, S]
            sin = np.asarray(jnp.sin(emb)).T
    except Exception:
        inv = 1.0 / (10000.0 ** (np.arange(0, P, 2, dtype=np.float64) / P))
        t = np.arange(S, dtype=np.float64)
        freqs = np.outer(t, inv)
        emb = np.concatenate((freqs, freqs), axis=-1)
        cos = np.cos(emb).T.astype(np.float32)
        sin = np.sin(emb).T.astype(np.float32)

    cos2 = np.ascontiguousarray(np.tile(cos, (1, B)).astype(np.float32))
    sin_z = np.concatenate([sin[0:64], -sin[64:128]], axis=0)
    sin2 = np.ascontiguousarray(np.tile(sin_z, (1, B)).astype(np.float32))
    return cos2, sin2


_NC_CACHE = None


def _in_maps(x, Wq, Wk, Wv, Wo):
    xT = np.ascontiguousarray(x.reshape(TOK, DIM).T).astype(np.float32)
    cosT, sinZ = _rope_tables()
    ones = np.ones((P, P), dtype=np.float32)
    maps = []
    for c in range(N_CORES):
        cs = slice(c * INNER_C, (c + 1) * INNER_C)
        maps.append({
            "xT": xT,
            "wq": np.ascontiguousarray(Wq[:, cs]).astype(np.float32),
            "wk": np.ascontiguousarray(Wk[:, cs]).astype(np.float32),
            "wv": np.ascontiguousarray(Wv[:, cs]).astype(np.float32),
            "wo": np.ascontiguousarray(Wo[cs, :]).astype(np.float32),
            "cosT": cosT,
            "sinZ": sinZ,
            "ones": ones,
        })
    return maps


def kernel(x, Wq, Wk, Wv, Wo):
    global _NC_CACHE
    assert x.shape == (B, S, DIM)
    if _NC_CACHE is None:
        _NC_CACHE = _build()
    res = run_bass_kernel_spmd(_NC_CACHE, _in_maps(x, Wq, Wk, Wv, Wo),
                               core_ids=list(range(N_CORES)), trace=False)
    out = res.results[0]["o_part"].astype(np.float64)
    for c in range(1, N_CORES):
        out += res.results[c]["o_part"]
    return out.astype(np.float32).reshape(B, S, DIM)


# revision 18
# speedup vs baseline: 2.9077x; 1.0609x over previous
"""Trainium2 Bass kernel for nn_Attention_11287174054323.

Full attention layer: QKV projections + RoPE + softmax attention + output
projection.  B=2, S=2048, DIM=2048, 16 heads x 128 head_dim, fp32.

Sharding: tensor-parallel over heads across 8 NeuronCores (2 heads/core).
Each core computes q/k/v projections for its head slice, full attention for
its heads, and a partial output projection (row slice of Wo); the host sums
the 8 partials.

Per-core layout strategy:
  - x is passed pre-transposed (xT [DIM, B*S]) so projections can contract
    over DIM on the partition axis.
  - Q^T/K^T are produced in [head_dim, token] layout; RoPE is fused into the
    PSUM eviction (rotate-half via cross-partition-write multiplies).
  - Scores are computed transposed (S^T = K @ Q^T), softmax-normalization is
    deferred past the A^T = V^T @ exp(S^T) accumulation (linearity); the
    denominator is accumulated with M=1 ones-matmuls in PSUM, broadcast with
    a K=1 matmul, inverted with reciprocal_approx_fast.
  - Matmuls run in float32r (tf32-like, ~13-bit mantissa, 4x fp32 rate).
  - PSUM->SBUF evictions that would crowd the vector engine go to the scalar
    engine (ACT) in phases where it is otherwise idle.
"""

import sys

sys.path.insert(0, "/opt/trn_rl_repo")

import numpy as np

import concourse.tile as tile
import concourse.mybir as mybir
from concourse import bacc
from concourse.bass_utils import run_bass_kernel_spmd

P = 128
B, S, DIM = 2, 2048, 2048
TOK = B * S                     # 4096 tokens
HEADS_PER_CORE = 2
INNER_C = HEADS_PER_CORE * P    # 256 per-core inner dim
KC = DIM // P                   # 16 contraction chunks
TC = 512                        # phase-1 token chunk
NTC = TOK // TC                 # 8
IC = 512                        # attention i-chunk (queries)
NIC = S // IC                   # 4 per (batch, head)
NJC = S // P                    # 16 key chunks per (batch, head)
SCALE = float(P) ** -0.5

F32 = mybir.dt.float32
MM = mybir.dt.float32r          # matmul dtype (float32r or float32)

N_CORES = 8
Mul = mybir.AluOpType.mult


def _build():
    nc = bacc.Bacc("TRN2", target_bir_lowering=False)

    xT_d = nc.dram_tensor("xT", [DIM, TOK], MM, kind="ExternalInput")
    wq_d = nc.dram_tensor("wq", [DIM, INNER_C], MM, kind="ExternalInput")
    wk_d = nc.dram_tensor("wk", [DIM, INNER_C], MM, kind="ExternalInput")
    wv_d = nc.dram_tensor("wv", [DIM, INNER_C], MM, kind="ExternalInput")
    wo_d = nc.dram_tensor("wo", [INNER_C, DIM], MM, kind="ExternalInput")
    cos_d = nc.dram_tensor("cosT", [P, TOK], F32, kind="ExternalInput")
    sin_d = nc.dram_tensor("sinZ", [P, TOK], F32, kind="ExternalInput")
    ones_d = nc.dram_tensor("ones", [P, P], MM, kind="ExternalInput")
    o_d = nc.dram_tensor("o_part", [TOK, DIM], F32, kind="ExternalOutput")

    xT_r = xT_d.rearrange("(c p) t -> p c t", p=P)

    with tile.TileContext(nc) as tc:
        with tc.tile_pool(name="persist", bufs=1) as persist, \
             tc.tile_pool(name="dram", bufs=1, space="DRAM") as dram:
            qt = [persist.tile([P, TOK], MM, tag=f"qt{h}", name=f"qt{h}")
                  for h in range(2)]
            kt = [persist.tile([P, TOK], MM, tag=f"kt{h}", name=f"kt{h}")
                  for h in range(2)]
            ones_t = persist.tile([P, P], MM, tag="ones")
            nc.sync.dma_start(ones_t[:], ones_d[:])
            v_dram = dram.tile([TOK, INNER_C], MM)

            # ---------------- Phase 1: QKV projections + RoPE ----------
            with tc.tile_pool(name="w1", bufs=1) as wpool, \
                 tc.tile_pool(name="x1", bufs=2) as xpool, \
                 tc.tile_pool(name="ev1", bufs=2) as evpool, \
                 tc.tile_pool(name="ps1", bufs=1, space="PSUM") as psum1:
                wq_t = wpool.tile([P, KC, INNER_C], MM, tag="wq")
                wk_t = wpool.tile([P, KC, INNER_C], MM, tag="wk")
                wv_t = wpool.tile([P, KC, INNER_C], MM, tag="wv")
                # critical path first: wq + chunk-0 activations, then wk/wv
                nc.sync.dma_start(wq_t[:], wq_d.rearrange("(c p) m -> p c m", p=P))
                xt0 = xpool.tile([P, KC, TC], MM, tag="xt")
                nc.sync.dma_start(xt0[:], xT_r[:, :, 0:TC])
                cos0 = evpool.tile([P, TC], F32, tag="cos")
                sin0 = evpool.tile([P, TC], F32, tag="sin")
                nc.sync.dma_start(cos0[:], cos_d[:, 0:TC])
                nc.sync.dma_start(sin0[:], sin_d[:, 0:TC])
                nc.sync.dma_start(wk_t[:], wk_d.rearrange("(c p) m -> p c m", p=P))
                nc.sync.dma_start(wv_t[:], wv_d.rearrange("(c p) m -> p c m", p=P))

                for tcn in range(NTC):
                    tsl = slice(tcn * TC, (tcn + 1) * TC)
                    if tcn == 0:
                        xt, cos_t, sin_t = xt0, cos0, sin0
                    else:
                        xt = xpool.tile([P, KC, TC], MM, tag="xt")
                        nc.sync.dma_start(xt[:], xT_r[:, :, tsl])
                        cos_t = evpool.tile([P, TC], F32, tag="cos")
                        sin_t = evpool.tile([P, TC], F32, tag="sin")
                        nc.sync.dma_start(cos_t[:], cos_d[:, tsl])
                        nc.sync.dma_start(sin_t[:], sin_d[:, tsl])

                    # Q^T / K^T chunks with fused RoPE eviction
                    for wt, dsts, nm in ((wq_t, qt, "q"), (wk_t, kt, "k")):
                        for m in range(2):
                            ps = psum1.tile([P, TC], F32, tag=f"ps_{nm}{m}")
                            for kc in range(KC):
                                nc.tensor.matmul(
                                    ps[:], wt[:, kc, m * P:(m + 1) * P],
                                    xt[:, kc, :],
                                    start=(kc == 0), stop=(kc == KC - 1))
                            # rope: dst = ps*cos + rotate_half(ps)*sin
                            tcos = evpool.tile([P, TC], F32, tag="tcos")
                            nc.vector.tensor_mul(tcos[:], ps[:], cos_t[:])
                            tsin = evpool.tile([P, TC], F32, tag="tsin")
                            nc.vector.scalar_tensor_tensor(
                                tsin[0:64, :], ps[64:128, :], 1.0,
                                sin_t[64:128, :], Mul, Mul)
                            nc.vector.scalar_tensor_tensor(
                                tsin[64:128, :], ps[0:64, :], 1.0,
                                sin_t[0:64, :], Mul, Mul)
                            nc.vector.tensor_add(dsts[m][:, tsl], tcos[:],
                                                 tsin[:])

                    # V chunks (tokens on partitions) -> DRAM scratch
                    for m in range(TC // P):
                        ps = psum1.tile([P, INNER_C], F32, tag=f"ps_v{m}")
                        for kc in range(KC):
                            nc.tensor.matmul(
                                ps[:], xt[:, kc, m * P:(m + 1) * P],
                                wv_t[:, kc, :],
                                start=(kc == 0), stop=(kc == KC - 1))
                        vst = evpool.tile([P, INNER_C], MM, tag="vst")
                        nc.scalar.copy(vst[:], ps[:])
                        r0 = tcn * TC + m * P
                        nc.sync.dma_start(v_dram[r0:r0 + P, :], vst[:])

            # ---------- Phases 2+3: attention + output projection ------
            with tc.tile_pool(name="at", bufs=1) as atpool, \
                 tc.tile_pool(name="vbh", bufs=2) as vpool, \
                 tc.tile_pool(name="e2", bufs=4) as epool, \
                 tc.tile_pool(name="sm2", bufs=2) as smpool, \
                 tc.tile_pool(name="p3", bufs=1) as p3pool, \
                 tc.tile_pool(name="st3", bufs=3) as stpool, \
                 tc.tile_pool(name="ps2", bufs=1, space="PSUM") as psum2, \
                 tc.tile_pool(name="ps3", bufs=2, space="PSUM") as psum3:
                at = [atpool.tile([P, TOK], MM, tag=f"at{h}", name=f"at{h}")
                      for h in range(2)]
                wo_t = p3pool.tile([P, 2, DIM], MM, tag="wo")
                nc.sync.dma_start(wo_t[:],
                                  wo_d.rearrange("(h p) e -> p h e", p=P))

                def load_vbh(b, h):
                    boff = b * S
                    vbh = vpool.tile([P, NJC, P], MM, tag="vbh")
                    nc.sync.dma_start(
                        vbh[:],
                        v_dram[boff:boff + S, h * P:(h + 1) * P]
                        .rearrange("(c p) d -> p c d", p=P))
                    return vbh

                def attn_ic(b, h, icn, vbh):
                    """One 512-query chunk of attention for (batch, head)."""
                    boff = b * S
                    isl = slice(boff + icn * IC, boff + (icn + 1) * IC)
                    ps_at = psum2.tile([P, IC], F32, tag="ps_at", bufs=2)
                    acc = smpool.tile([P, IC], MM, tag="acc")
                    # software-pipelined S -> exp -> (A, colsum) chain
                    es = [None] * NJC

                    def s_step(jc):
                        jsl = slice(boff + jc * P, boff + (jc + 1) * P)
                        ps_s = psum2.tile([P, IC], F32, tag="ps_s", bufs=2)
                        nc.tensor.matmul(ps_s[:], kt[h][:, jsl], qt[h][:, isl],
                                         start=True, stop=True)
                        e = epool.tile([P, IC], MM, tag="e")
                        nc.scalar.activation(
                            e[:], ps_s[:], mybir.ActivationFunctionType.Exp,
                            scale=SCALE)
                        es[jc] = e

                    def a_step(jc):
                        e = es[jc]
                        nc.tensor.matmul(ps_at[:], vbh[:, jc, :], e[:],
                                         start=(jc == 0), stop=(jc == NJC - 1))
                        if jc == 0:
                            nc.vector.tensor_copy(acc[:], e[:].bitcast(F32))
                        else:
                            nc.vector.tensor_add(acc[:], acc[:].bitcast(F32),
                                                 e[:].bitcast(F32))

                    s_step(0)
                    for jc in range(NJC):
                        if jc + 1 < NJC:
                            s_step(jc + 1)
                        a_step(jc)

                    ps_bc = psum2.tile([P, IC], F32, tag="ps_bc", bufs=1)
                    nc.tensor.matmul(ps_bc[:], ones_t[:], acc[:],
                                     start=True, stop=True)
                    recip = smpool.tile([P, IC], F32, tag="recip")
                    nc.vector.reciprocal_approx_fast(recip[:], ps_bc[:])
                    nc.vector.tensor_mul(at[h][:, isl], ps_at[:], recip[:])

                def ph3_tn(tn):
                    """One 128-token chunk of the output projection."""
                    stage = stpool.tile([P, DIM], F32, tag="stage")
                    for en in range(DIM // IC):
                        ps = psum3.tile([P, IC], F32, tag="ps_o")
                        esl = slice(en * IC, (en + 1) * IC)
                        for h in range(2):
                            nc.tensor.matmul(
                                ps[:], at[h][:, tn * P:(tn + 1) * P],
                                wo_t[:, h, esl],
                                start=(h == 0), stop=(h == 1))
                        nc.scalar.copy(stage[:, esl], ps[:])
                    nc.sync.dma_start(o_d[tn * P:(tn + 1) * P, :], stage[:])

                # batch 0 attention
                for h in range(2):
                    vbh = load_vbh(0, h)
                    for icn in range(NIC):
                        attn_ic(0, h, icn, vbh)
                # batch 1 attention; interleave batch-0 out-projection during
                # h=0 and batch-1 chunks (as their at-slices complete) in h=1
                vbh = load_vbh(1, 0)
                for icn in range(NIC):
                    attn_ic(1, 0, icn, vbh)
                    for k in range(4):
                        ph3_tn(icn * 4 + k)
                vbh = load_vbh(1, 1)
                for icn in range(NIC):
                    attn_ic(1, 1, icn, vbh)
                    if icn > 0:
                        for k in range(4):
                            ph3_tn(16 + (icn - 1) * 4 + k)
                for k in range(4):
                    ph3_tn(16 + (NIC - 1) * 4 + k)

    nc.finalize()
    return nc


def _rope_tables():
    """cos/sin tables in [head_dim, token] layout, matching the reference's
    f32 computation (jax on CPU when available).

    sinZ rows 0:64 hold +sin (multiplied against q[d-64] to produce rows
    64:128 of the rotation term) and rows 64:128 hold -sin (multiplied
    against q[d+64] to produce rows 0:64); both halves of the underlying
    sin table are identical (emb = concat(freqs, freqs)).
    """
    try:
        import jax
        import jax.numpy as jnp
        cpu = jax.devices("cpu")[0]
        with jax.default_device(cpu):
            inv = 1.0 / (10000.0 ** (
                jnp.arange(0, P, 2, dtype=jnp.float32) / P))
            t = jnp.arange(S, dtype=jnp.float32)
            freqs = jnp.einsum("i,j->ij", t, inv)          # [S, 64]
            emb = jnp.concatenate((freqs, freqs), axis=-1)  # [S, 128]
            cos = np.asarray(jnp.cos(emb)).T                # [128, S]
            sin = np.asarray(jnp.sin(emb)).T
    except Exception:
        inv = 1.0 / (10000.0 ** (np.arange(0, P, 2, dtype=np.float64) / P))
        t = np.arange(S, dtype=np.float64)
        freqs = np.outer(t, inv)
        emb = np.concatenate((freqs, freqs), axis=-1)
        cos = np.cos(emb).T.astype(np.float32)
        sin = np.sin(emb).T.astype(np.float32)

    cos2 = np.ascontiguousarray(np.tile(cos, (1, B)).astype(np.float32))
    sin_z = np.concatenate([sin[0:64], -sin[64:128]], axis=0)
    sin2 = np.ascontiguousarray(np.tile(sin_z, (1, B)).astype(np.float32))
    return cos2, sin2


_NC_CACHE = None


def _in_maps(x, Wq, Wk, Wv, Wo):
    xT = np.ascontiguousarray(x.reshape(TOK, DIM).T).astype(np.float32)
    cosT, sinZ = _rope_tables()
    ones = np.ones((P, P), dtype=np.float32)
    maps = []
    for c in range(N_CORES):
        cs = slice(c * INNER_C, (c + 1) * INNER_C)
        maps.append({
            "xT": xT,
            "wq": np.ascontiguousarray(Wq[:, cs]).astype(np.float32),
            "wk": np.ascontiguousarray(Wk[:, cs]).astype(np.float32),
            "wv": np.ascontiguousarray(Wv[:, cs]).astype(np.float32),
            "wo": np.ascontiguousarray(Wo[cs, :]).astype(np.float32),
            "cosT": cosT,
            "sinZ": sinZ,
            "ones": ones,
        })
    return maps


def kernel(x, Wq, Wk, Wv, Wo):
    global _NC_CACHE
    assert x.shape == (B, S, DIM)
    if _NC_CACHE is None:
        _NC_CACHE = _build()
    res = run_bass_kernel_spmd(_NC_CACHE, _in_maps(x, Wq, Wk, Wv, Wo),
                               core_ids=list(range(N_CORES)), trace=False)
    out = res.results[0]["o_part"].astype(np.float64)
    for c in range(1, N_CORES):
        out += res.results[c]["o_part"]
    return out.astype(np.float32).reshape(B, S, DIM)
